# revision 1
# baseline (speedup 1.0000x reference)
"""Trainium2 Bass kernel for CohereAttention (B=2, S=2048, H=4096, 32Q/8KV heads, D=128).

Sharding: 8 cores = 2 batch groups x 4 tensor-parallel (head) ranks.
Core c: batch b = c // 4, tp rank t = c % 4.
  - owns q-heads [8t, 8t+8), kv-heads [2t, 2t+2) (GQA-aligned), w_o col slice
    [1024t, 1024(t+1)).
  - per-head attention output (transposed [d, s]) is AllGather'd across the 4
    ranks of the batch group, chunk-by-chunk (overlaps attention compute);
    o_proj then computes its 1024-column output slice with no all-reduce.

All matmuls run as float32r (TF32-like, full PE rate at moving-dim >= 256) on
fp32 data. RoPE is restructured host-side: q/k weight columns are permuted to
"neox" halves (evens then odds) so the on-device rotation is two block copies
plus elementwise ops against duplicated cos/sin tables.

Attention is computed in transposed score layout sT[k, q] so the exp'd scores
feed PV matmuls directly (no PE transposes); the softmax denominator comes from
a ones-vector matmul accumulated alongside PV, and normalization happens during
PSUM eviction.
"""

import numpy as np

import concourse.bass as bass
import concourse.mybir as mybir
from concourse import bacc
import concourse.tile as tile
from concourse.bass_utils import run_bass_kernel_spmd

# Problem constants (fixed by the task).
B, S, H = 2, 2048, 4096
NQ, NKV, D = 32, 8, 128
THETA = 10000.0
NCORES = 8
TP = 4                      # head-parallel group size
QH = NQ // TP               # 8 q heads per core
KH = NKV // TP              # 2 kv heads per core
REP = NQ // NKV             # 4
SCALE = float(D) ** -0.5
QC = QH * D                 # 1024 local q cols
KC = KH * D                 # 256 local k cols
OC = H // TP                # 1024 output cols per core
P = 128
KT = H // P                 # 32 contraction tiles for projections
AKT = NQ * D // P           # 32 contraction tiles for o_proj
NSB = S // P                # 16 seq blocks
SG = 512
NSG = S // SG               # 4 seq groups
F32 = mybir.dt.float32
F32R = mybir.dt.float32r
NCT = (QC + KC) // P        # 10 q/k col-tiles per core
RG = [[0, 1, 2, 3], [4, 5, 6, 7]]

Exp = mybir.ActivationFunctionType.Exp


def _r(ap):
    return ap.bitcast(F32R)


def build_program(no_collective=False, phase3_reads_att=False):
    """Emit the SPMD Bass program (same program for all 8 cores)."""
    nc = bacc.Bacc('TRN2', target_bir_lowering=False, debug=False, num_devices=NCORES)

    hidT = nc.dram_tensor("hidT", [H, S], F32, kind="ExternalInput")
    wqk = nc.dram_tensor("wqk", [H, QC + KC], F32, kind="ExternalInput")
    wv = nc.dram_tensor("wv", [H, KC], F32, kind="ExternalInput")
    wo = nc.dram_tensor("wo", [NQ * D, OC], F32, kind="ExternalInput")
    cosf = nc.dram_tensor("cosf", [P, S], F32, kind="ExternalInput")
    sins = nc.dram_tensor("sins", [P, S], F32, kind="ExternalInput")
    out = nc.dram_tensor("out", [S, OC], F32, kind="ExternalOutput")

    qkT_d = nc.dram_tensor("qkT_d", [QC + KC, S], F32)       # roped qT/kT
    v_d = nc.dram_tensor("v_d", [S, KC], F32)                # v natural
    att_d = nc.dram_tensor("att_d", [QH, P, S], F32)         # local attnT
    gath_d = nc.dram_tensor("gath_d", [QH, TP * P, S], F32)

    wqk_t = wqk.rearrange("(ko p) c -> p ko c", p=P)
    wv_t = wv.rearrange("(ko p) c -> p ko c", p=P)
    wo_t = wo.rearrange("(ko p) c -> p ko c", p=P)

    with tile.TileContext(nc) as tc:
        # ---------------- Phase 1: qkv projection + RoPE ----------------
        # Contraction (H=4096) is split in two halves so only half of
        # hidT's columns for the current seq half must be SBUF-resident;
        # partial products are accumulated in SBUF (acc/v_sb) across halves.
        with tc.tile_pool(name="ph1_hid", bufs=1) as hidp, \
             tc.tile_pool(name="ph1_w", bufs=2) as wp, \
             tc.tile_pool(name="ph1_wv", bufs=2) as wvp, \
             tc.tile_pool(name="ph1_cs", bufs=1) as csp, \
             tc.tile_pool(name="ph1_acc", bufs=1) as accp, \
             tc.tile_pool(name="ph1_rope", bufs=2) as rp, \
             tc.tile_pool(name="ph1_vsb", bufs=1) as vsp, \
             tc.tile_pool(name="ph1_ps", bufs=2, space="PSUM") as pp, \
             tc.tile_pool(name="ph1_psv", bufs=2, space="PSUM") as ppv:
            KH2 = KT // 2   # 16 k-tiles per contraction half
            for sg in range(2):          # seq halves of 1024
                sgs = slice(sg * 1024, (sg + 1) * 1024)
                cos_sb = csp.tile([P, 1024], F32, tag="cos")
                sin_sb = csp.tile([P, 1024], F32, tag="sin")
                nc.sync.dma_start(out=cos_sb[:], in_=cosf[:, sgs])
                nc.sync.dma_start(out=sin_sb[:], in_=sins[:, sgs])
                v_sb = vsp.tile([P, 8, KC], F32, tag="vacc")
                acc = accp.tile([P, NCT, 1024], F32, tag="acc")
                for kh in range(2):      # contraction halves (16 k-tiles)
                    hid_sb = hidp.tile([P, KH2, 1024], F32R, tag="hid")
                    for kt in range(KH2):
                        nc.sync.dma_start(
                            out=hid_sb[:, kt, :],
                            in_=hidT[(kh * KH2 + kt) * P:(kh * KH2 + kt + 1) * P, sgs].bitcast(F32R),
                        )
                    # q/k col-tiles: out[M=col, N=seq] = wqk.T @ hidT
                    for ct in range(NCT):
                        w_sb = wp.tile([P, KH2, P], F32R, tag="wqk")
                        for kt in range(KH2):
                            nc.sync.dma_start(
                                out=w_sb[:, kt, :],
                                in_=wqk_t[:, kh * KH2 + kt, ct * P:(ct + 1) * P].bitcast(F32R),
                            )
                        ps = pp.tile([P, 1024], F32, tag="ps", name=f"ps_{sg}_{kh}_{ct}")
                        for kt in range(KH2):
                            for nn in range(2):
                                nc.tensor.matmul(
                                    ps[:, nn * 512:(nn + 1) * 512],
                                    w_sb[:, kt, :],
                                    hid_sb[:, kt, nn * 512:(nn + 1) * 512],
                                    start=(kt == 0), stop=(kt == KH2 - 1),
                                )
                        for nn in range(2):
                            nsl = slice(nn * 512, (nn + 1) * 512)
                            if kh == 0:
                                nc.scalar.copy(acc[:, ct, nsl], ps[:, nsl])
                            else:
                                nc.vector.tensor_add(
                                    acc[:, ct, nsl], acc[:, ct, nsl], ps[:, nsl]
                                )
                    # v projection: out[M=seq, N=vcols] = hid @ wv
                    wv_sb = wvp.tile([P, KH2, KC], F32R, tag="wv")
                    for kt in range(KH2):
                        nc.sync.dma_start(
                            out=wv_sb[:, kt, :], in_=wv_t[:, kh * KH2 + kt, :].bitcast(F32R)
                        )
                    for sbl in range(8):
                        psv = ppv.tile(
                            [P, KC], F32, tag="psv", name=f"psv_{sg}_{kh}_{sbl}"
                        )
                        for kt in range(KH2):
                            nc.tensor.matmul(
                                psv[:],
                                hid_sb[:, kt, sbl * P:(sbl + 1) * P],
                                wv_sb[:, kt, :],
                                start=(kt == 0), stop=(kt == KH2 - 1),
                            )
                        if kh == 0:
                            nc.scalar.copy(v_sb[:, sbl, :], psv[:])
                        else:
                            nc.vector.tensor_add(v_sb[:, sbl, :], v_sb[:, sbl, :], psv[:])
                # RoPE on q/k (each col-tile is one whole head) + store
                for ct in range(NCT):
                    x = acc[:, ct, :]
                    tmp = rp.tile([P, 1024], F32, tag="tmp")
                    # rot(x) = [-x2; x1] (sign folded into sins rows)
                    nc.vector.tensor_copy(tmp[0:64, :], x[64:128, :])
                    nc.vector.tensor_copy(tmp[64:128, :], x[0:64, :])
                    t1 = rp.tile([P, 1024], F32, tag="t1")
                    nc.vector.tensor_mul(t1[:], x, cos_sb[:])
                    nc.vector.tensor_mul(tmp[:], tmp[:], sin_sb[:])
                    nc.vector.tensor_add(t1[:], t1[:], tmp[:])
                    nc.sync.dma_start(out=qkT_d[ct * P:(ct + 1) * P, sgs], in_=t1[:])
                for sbl in range(8):
                    nc.sync.dma_start(
                        out=v_d[sg * 1024 + sbl * P:sg * 1024 + (sbl + 1) * P, :],
                        in_=v_sb[:, sbl, :],
                    )

        # ---------------- Phase 2: attention (transposed scores) ---------
        with tc.tile_pool(name="ph2_kv", bufs=1) as kvp, \
             tc.tile_pool(name="ph2_q", bufs=2) as qp, \
             tc.tile_pool(name="ph2_p", bufs=6) as ppl, \
             tc.tile_pool(name="ph2_o", bufs=2) as op, \
             tc.tile_pool(name="ph2_c", bufs=1) as cp, \
             tc.tile_pool(name="ph2_ps", bufs=2, space="PSUM") as sp, \
             tc.tile_pool(name="ph2_pacc", bufs=2, space="PSUM") as ap_, \
             tc.tile_pool(name="ph2_pden", bufs=2, space="PSUM") as dp, \
             tc.tile_pool(name="ph2_pbc", bufs=1, space="PSUM") as bp:
            ones_f = cp.tile([P, 1], F32, tag="ones_f")
            nc.vector.memset(ones_f[:], 1.0)
            ones_sb = cp.tile([P, 1], F32R, tag="ones")
            nc.sync.dma_start(out=ones_sb[:], in_=ones_f[:].bitcast(F32R))
            ones_row = cp.tile([1, P], F32, tag="onesr")
            nc.vector.memset(ones_row[:], 1.0)
            for kv in range(KH):
                kT_sb = kvp.tile([P, S], F32R, tag="kT")
                nc.sync.dma_start(out=kT_sb[:], in_=qkT_d[QC + kv * P:QC + (kv + 1) * P, :].bitcast(F32R))
                vn_sb = kvp.tile([P, NSB, P], F32R, tag="vn")
                vd_r = v_d.rearrange("(nb p) c -> p nb c", p=P)
                nc.sync.dma_start(out=vn_sb[:], in_=vd_r[:, :, kv * P:(kv + 1) * P].bitcast(F32R))
                for qi in range(REP):
                    qh = kv * REP + qi
                    qT_sb = qp.tile([P, S], F32R, tag="qT")
                    nc.sync.dma_start(out=qT_sb[:], in_=qkT_d[qh * P:(qh + 1) * P, :].bitcast(F32R))
                    for j in range(NSG):
                        ncb = 4 * j + 4
                        att_ps = ap_.tile([P, SG], F32, tag="att", name=f"att_{qh}_{j}")
                        den_ps = dp.tile([1, SG], F32, tag="den", name=f"den_{qh}_{j}")
                        qs = slice(j * SG, (j + 1) * SG)
                        for c in range(ncb):
                            s_ps = sp.tile([P, SG], F32, tag="s", name=f"s_{qh}_{j}_{c}")
                            nc.tensor.matmul(
                                s_ps[:],
                                kT_sb[:, c * P:(c + 1) * P],
                                qT_sb[:, qs],
                                start=True, stop=True,
                            )
                            p_sb = ppl.tile([P, SG], F32R, tag="p", name=f"p_{qh}_{j}_{c}")
                            nc.scalar.activation(p_sb[:], s_ps[:], Exp, scale=SCALE)
                            if c >= 4 * j:
                                # zero p where k > q (causal), diagonal chunk
                                r = c - 4 * j
                                nc.gpsimd.affine_select(
                                    out=p_sb[:], in_=p_sb[:],
                                    compare_op=mybir.AluOpType.is_ge,
                                    fill=0.0, base=-(P * r),
                                    pattern=[[1, SG]], channel_multiplier=-1,
                                )
                            nc.tensor.matmul(
                                den_ps[:], ones_sb[:], p_sb[:],
                                start=(c == 0), stop=(c == ncb - 1),
                            )
                            nc.tensor.matmul(
                                att_ps[:], vn_sb[:, c, :], p_sb[:],
                                start=(c == 0), stop=(c == ncb - 1),
                            )
                        rinv = op.tile([1, SG], F32, tag="rinv")
                        nc.vector.reciprocal(rinv[:], den_ps[:])
                        rb_ps = bp.tile([P, SG], F32, tag="rb", name=f"rb_{qh}_{j}")
                        nc.tensor.matmul(
                            rb_ps[:], ones_row[:], rinv[:],
                            start=True, stop=True,
                        )
                        rb_sb = op.tile([P, SG], F32, tag="rb_sb")
                        nc.scalar.copy(rb_sb[:], rb_ps[:])
                        att_sb = op.tile([P, SG], F32, tag="att_sb")
                        nc.vector.tensor_mul(att_sb[:], att_ps[:], rb_sb[:])
                        nc.sync.dma_start(out=att_d[qh, :, qs], in_=att_sb[:])
                    if no_collective:
                        nc.sync.dma_start(out=gath_d[qh, 0:P, :], in_=att_d[qh])
                    else:
                        nc.gpsimd.collective_compute(
                            "AllGather", mybir.AluOpType.bypass,
                            replica_groups=RG,
                            ins=[att_d[qh].opt()],
                            outs=[gath_d[qh].opt()],
                        )

        # ---------------- Phase 3: o_proj (column slice) -----------------
        with tc.tile_pool(name="ph3_wo", bufs=1) as wop, \
             tc.tile_pool(name="ph3_g", bufs=2) as gp, \
             tc.tile_pool(name="ph3_o", bufs=3) as oop, \
             tc.tile_pool(name="ph3_ps", bufs=4, space="PSUM") as p3:
            wo_sb = wop.tile([P, AKT, OC], F32R, tag="wo")
            for kt in range(AKT):
                nc.sync.dma_start(out=wo_sb[:, kt, :], in_=wo_t[:, kt, :].bitcast(F32R))
            for sb in range(NSB):
                g_sb = gp.tile([P, QH, TP, P], F32R, tag="g")
                for h in range(QH):
                    if phase3_reads_att:
                        for r in range(TP):
                            nc.sync.dma_start(
                                out=g_sb[:, h, r, :],
                                in_=att_d[h][:, sb * P:(sb + 1) * P].bitcast(F32R),
                            )
                    else:
                        gd = gath_d[h].rearrange("(r p) s -> p r s", p=P)
                        nc.sync.dma_start(
                            out=g_sb[:, h, :, :], in_=gd[:, :, sb * P:(sb + 1) * P].bitcast(F32R)
                        )
                for oc in range(2):
                    ps = p3.tile([P, 512], F32, tag="o", name=f"o_{sb}_{oc}")
                    for h in range(QH):
                        for r in range(TP):
                            kt = 4 * h + r
                            nc.tensor.matmul(
                                ps[:],
                                g_sb[:, h, r, :],
                                wo_sb[:, kt, oc * 512:(oc + 1) * 512],
                                start=(kt == 0), stop=(kt == AKT - 1),
                            )
                    o_sb = oop.tile([P, 512], F32, tag="osb")
                    nc.scalar.copy(o_sb[:], ps[:])
                    nc.sync.dma_start(
                        out=out[sb * P:(sb + 1) * P, oc * 512:(oc + 1) * 512],
                        in_=o_sb[:],
                    )
    nc.compile()
    return nc


def _prep_inputs(hidden_states, w_qkv, w_o, positions):
    """Host-side sharding + weight permutation. Returns per-core in_maps."""
    hidden_states = np.asarray(hidden_states, dtype=np.float32)
    w_qkv = np.asarray(w_qkv, dtype=np.float32)
    w_o = np.asarray(w_o, dtype=np.float32)
    positions = np.asarray(positions)

    # neox permutation of q/k head columns (evens then odds within each head)
    perm = np.concatenate([np.arange(0, D, 2), np.arange(1, D, 2)])
    wq_all = w_qkv[:, :NQ * D].reshape(H, NQ, D)[:, :, perm]      # [H, NQ, D]
    wk_all = w_qkv[:, NQ * D:(NQ + NKV) * D].reshape(H, NKV, D)[:, :, perm]
    wv_all = w_qkv[:, (NQ + NKV) * D:].reshape(H, NKV, D)

    # o_proj row permutation to match chunked AllGather order:
    # k-tile (h, r) holds global head 8r + h.
    head_order = np.array([8 * r + h for h in range(QH) for r in range(TP)])
    wo_perm = w_o.reshape(NQ, D, H)[head_order]                   # [32, D, H]

    # cos/sin tables, duplicated halves; sin top rows negated.
    inv_freq = 1.0 / (THETA ** (np.arange(0, D, 2, dtype=np.float32) / D))
    in_maps = []
    for c in range(NCORES):
        b, t = c // TP, c % TP
        freqs = positions[b].astype(np.float32)[None, :] * inv_freq[:, None]
        cos = np.cos(freqs)                                       # [64, S]
        sin = np.sin(freqs)
        cosf = np.concatenate([cos, cos], axis=0).astype(np.float32)
        sins = np.concatenate([-sin, sin], axis=0).astype(np.float32)

        wq = wq_all[:, 8 * t:8 * t + 8].reshape(H, QC)
        wk = wk_all[:, 2 * t:2 * t + 2].reshape(H, KC)
        wv = wv_all[:, 2 * t:2 * t + 2].reshape(H, KC)
        in_maps.append({
            "hidT": np.ascontiguousarray(hidden_states[b].T),
            "wqk": np.ascontiguousarray(np.concatenate([wq, wk], axis=1)),
            "wv": np.ascontiguousarray(wv),
            "wo": np.ascontiguousarray(
                wo_perm[:, :, 1024 * t:1024 * (t + 1)].reshape(NQ * D, OC)
            ),
            "cosf": cosf,
            "sins": sins,
        })
    return in_maps


_NC_CACHE = {}


def kernel(hidden_states, w_qkv, w_o, positions, _trace=False):
    if "nc" not in _NC_CACHE:
        _NC_CACHE["nc"] = build_program()
    nc = _NC_CACHE["nc"]
    in_maps = _prep_inputs(hidden_states, w_qkv, w_o, positions)
    res = run_bass_kernel_spmd(nc, in_maps, list(range(NCORES)), trace=_trace)
    out_full = np.empty((B, S, H), dtype=np.float32)
    for c in range(NCORES):
        b, t = c // TP, c % TP
        out_full[b, :, 1024 * t:1024 * (t + 1)] = res.results[c]["out"]
    if _trace:
        kernel.last_exec_time_ns = res.exec_time_ns
        kernel.last_profile = res
    return out_full



# revision 4
# speedup vs baseline: 1.3156x; 1.3156x over previous
"""Trainium2 Bass kernel for CohereAttention (B=2, S=2048, H=4096, 32Q/8KV heads, D=128).

Sharding: 8 cores = 2 batch groups x 4 tensor-parallel (head) ranks.
Core c: batch b = c // 4, tp rank t = c % 4; owns q-heads [8t, 8t+8),
kv-heads [2t, 2t+2), w_o column slice [1024t, 1024(t+1)).

v2 layout/pipeline rewrite over the f32r baseline:
  - all matmul operands bf16 (PSUM accumulation stays f32); rel-err budget
    is 2e-2, measured impact ~1e-3.
  - every DRAM tensor is laid out host-side as "blob" tiles whose
    per-partition lines are 1KB-64KB contiguous, so DMA descriptors are
    large (the f32r baseline moved w_qkv with 512B descriptors at ~10GB/s).
  - roped qT/kT and v never round-trip through DRAM: phase 1 writes them
    straight into SBUF-resident tiles that attention reads.
  - phase 1 runs in 4 seq chunks of 512; attention q-group j only needs
    chunks <= j, so attention group j is emitted right after chunk j and
    the PE never idles at the phase boundary.
  - attention outputs AllGather per seq group (4 collectives of 0.5MB)
    instead of per head at the end; o_proj runs last, consuming gathered
    groups that have long since landed (only the final group's collective
    is partially exposed).
  - causal masking is a precomputed-mask multiply on the Vector engine;
    the Scalar (Act) engine runs only Exp (no act-table thrash), GpSimd
    only triggers collectives.
"""

import numpy as np
import ml_dtypes

import concourse.bass as bass
import concourse.mybir as mybir
from concourse import bacc
import concourse.tile as tile
from concourse.bass_utils import run_bass_kernel_spmd

# Problem constants (fixed by the task).
B, S, H = 2, 2048, 4096
NQ, NKV, D = 32, 8, 128
THETA = 10000.0
NCORES = 8
TP = 4                      # head-parallel group size
QH = NQ // TP               # 8 q heads per core
KH = NKV // TP              # 2 kv heads per core
REP = NQ // NKV             # 4
SCALE = float(D) ** -0.5
P = 128
KT = H // P                 # 32 contraction tiles for projections
OC = H // TP                # 1024 output cols per core
CW = 512                    # seq chunk width
SGC = S // CW               # 4 seq chunks / attention q-groups
NCT = QH + KH               # 10 q/k col-tiles per core
NVB = S // P                # 16 k/v seq blocks
F32 = mybir.dt.float32
BF16 = mybir.dt.bfloat16
RG = [[0, 1, 2, 3], [4, 5, 6, 7]]

Exp = mybir.ActivationFunctionType.Exp


def build_program():
    """Emit the SPMD Bass program (same program for all 8 cores)."""
    nc = bacc.Bacc('TRN2', target_bir_lowering=False, debug=False, num_devices=NCORES)

    # Blob inputs (host pre-tiled; per-partition lines are contiguous in DRAM).
    hidb = nc.dram_tensor("hidb", [SGC, P, KT, CW], BF16, kind="ExternalInput")
    wqkb = nc.dram_tensor("wqkb", [NCT, P, KT, P], BF16, kind="ExternalInput")
    wvb = nc.dram_tensor("wvb", [P, KT, KH * P], BF16, kind="ExternalInput")
    wob = nc.dram_tensor("wob", [P, KT, OC], BF16, kind="ExternalInput")
    cosf = nc.dram_tensor("cosf", [P, S], F32, kind="ExternalInput")
    sins = nc.dram_tensor("sins", [P, S], F32, kind="ExternalInput")
    maskb = nc.dram_tensor("maskb", [P, REP, CW], BF16, kind="ExternalInput")
    out = nc.dram_tensor("out", [S, OC], F32, kind="ExternalOutput")

    att_d = nc.dram_tensor("att_d", [SGC, QH, P, CW], BF16)
    gath_d = nc.dram_tensor("gath_d", [SGC, TP, QH, P, CW], BF16)

    with tile.TileContext(nc) as tc:
        with tc.tile_pool(name="persist", bufs=1) as pers, \
             tc.tile_pool(name="att_p", bufs=4) as ppl, \
             tc.tile_pool(name="att_o", bufs=3) as op, \
             tc.tile_pool(name="ps_s", bufs=2, space="PSUM") as sp, \
             tc.tile_pool(name="ps_att", bufs=2, space="PSUM") as ap_, \
             tc.tile_pool(name="ps_den", bufs=1, space="PSUM") as dp:
            # SBUF-resident across phases.
            qT = pers.tile([P, QH, S], BF16, tag="qT")
            kTt = pers.tile([P, KH, S], BF16, tag="kT")
            vsb = pers.tile([P, NVB, KH * P], BF16, tag="v")
            mask_sb = pers.tile([P, REP, CW], BF16, tag="mask")
            nc.sync.dma_start(out=mask_sb[:], in_=maskb[:])
            ones_c = pers.tile([P, 1], BF16, tag="ones_c")
            nc.vector.memset(ones_c[:], 1.0)
            ones_r = pers.tile([1, P], BF16, tag="ones_r")
            nc.vector.memset(ones_r[:], 1.0)

            def attention_group(j):
                ncb = 4 * j + 4
                for h in range(QH):
                    kv = h // REP
                    att_ps = ap_.tile([P, CW], F32, tag="att", name=f"att_{j}_{h}")
                    den_ps = dp.tile([1, CW], F32, tag="den", name=f"den_{j}_{h}")
                    for ci in range(ncb):
                        s_ps = sp.tile([P, CW], F32, tag="s", name=f"s_{j}_{h}_{ci}")
                        nc.tensor.matmul(
                            s_ps[:],
                            kTt[:, kv, ci * P:(ci + 1) * P],
                            qT[:, h, j * CW:(j + 1) * CW],
                            start=True, stop=True,
                        )
                        p_sb = ppl.tile([P, CW], BF16, tag="p", name=f"p_{j}_{h}_{ci}")
                        nc.scalar.activation(p_sb[:], s_ps[:], Exp, scale=SCALE)
                        r = ci - 4 * j
                        if r >= 0:
                            # diagonal chunk: zero where k > q
                            nc.vector.tensor_mul(p_sb[:], p_sb[:], mask_sb[:, r, :])
                        nc.tensor.matmul(
                            den_ps[:], ones_c[:], p_sb[:],
                            start=(ci == 0), stop=(ci == ncb - 1),
                        )
                        nc.tensor.matmul(
                            att_ps[:], vsb[:, ci, kv * P:(kv + 1) * P], p_sb[:],
                            start=(ci == 0), stop=(ci == ncb - 1),
                        )
                    rinv = op.tile([1, CW], F32, tag="rinv")
                    nc.vector.reciprocal(rinv[:], den_ps[:])
                    rinv_b = op.tile([1, CW], BF16, tag="rinvb")
                    nc.vector.tensor_copy(rinv_b[:], rinv[:])
                    rb_ps = sp.tile([P, CW], F32, tag="s", name=f"rb_{j}_{h}")
                    nc.tensor.matmul(rb_ps[:], ones_r[:], rinv_b[:], start=True, stop=True)
                    rb_sb = op.tile([P, CW], F32, tag="rb_sb")
                    nc.vector.tensor_copy(rb_sb[:], rb_ps[:])
                    att_sb = op.tile([P, CW], BF16, tag="att_sb")
                    nc.vector.tensor_mul(att_sb[:], att_ps[:], rb_sb[:])
                    nc.sync.dma_start(out=att_d[j, h], in_=att_sb[:])
                nc.gpsimd.collective_compute(
                    "AllGather", mybir.AluOpType.bypass,
                    replica_groups=RG,
                    ins=[att_d[j].opt()],
                    outs=[gath_d[j].opt()],
                )

            # ---------------- Phase 1 (+ interleaved attention) ----------
            with tc.tile_pool(name="ph1_hid", bufs=2) as hidp, \
                 tc.tile_pool(name="ph1_w", bufs=2) as wp, \
                 tc.tile_pool(name="ph1_wv", bufs=1) as wvp, \
                 tc.tile_pool(name="ph1_cs", bufs=2) as csp, \
                 tc.tile_pool(name="ph1_rope", bufs=2) as rp, \
                 tc.tile_pool(name="ph1_ps", bufs=2, space="PSUM") as pp, \
                 tc.tile_pool(name="ph1_psv", bufs=1, space="PSUM") as ppv:
                wv_sb = wvp.tile([P, KT, KH * P], BF16, tag="wv")
                nc.sync.dma_start(out=wv_sb[:], in_=wvb[:])
                for c in range(SGC):
                    cs = slice(c * CW, (c + 1) * CW)
                    hid_sb = hidp.tile([P, KT, CW], BF16, tag="hid")
                    nc.sync.dma_start(out=hid_sb[:], in_=hidb[c])
                    cos_sb = csp.tile([P, CW], F32, tag="cos")
                    sin_sb = csp.tile([P, CW], F32, tag="sin")
                    nc.sync.dma_start(out=cos_sb[:], in_=cosf[:, cs])
                    nc.sync.dma_start(out=sin_sb[:], in_=sins[:, cs])
                    for ct in range(NCT):
                        w_sb = wp.tile([P, KT, P], BF16, tag="w")
                        nc.sync.dma_start(out=w_sb[:], in_=wqkb[ct])
                        ps = pp.tile([P, CW], F32, tag="ps", name=f"ps_{c}_{ct}")
                        for kt in range(KT):
                            nc.tensor.matmul(
                                ps[:], w_sb[:, kt, :], hid_sb[:, kt, :],
                                start=(kt == 0), stop=(kt == KT - 1),
                            )
                        # RoPE (neox halves): out = x*cos + rot(x)*sin,
                        # rot(x) = [-x2; x1] with sign folded into sins rows.
                        if ct < QH:
                            dst = qT[:, ct, cs]
                        else:
                            dst = kTt[:, ct - QH, cs]
                        t1 = rp.tile([P, CW], F32, tag="t1")
                        t2 = rp.tile([P, CW], F32, tag="t2")
                        nc.vector.tensor_mul(t1[:], ps[:], cos_sb[:])
                        nc.vector.tensor_mul(t2[0:64, :], ps[64:128, :], sin_sb[0:64, :])
                        nc.vector.tensor_mul(t2[64:128, :], ps[0:64, :], sin_sb[64:128, :])
                        nc.vector.tensor_add(dst, t1[:], t2[:])
                    # v projection for this chunk's 4 seq blocks
                    for sbl in range(4):
                        psv = ppv.tile([P, KH * P], F32, tag="psv",
                                       name=f"psv_{c}_{sbl}")
                        for kt in range(KT):
                            nc.tensor.matmul(
                                psv[:],
                                hid_sb[:, kt, sbl * P:(sbl + 1) * P],
                                wv_sb[:, kt, :],
                                start=(kt == 0), stop=(kt == KT - 1),
                            )
                        nc.vector.tensor_copy(vsb[:, c * 4 + sbl, :], psv[:])
                    if c < SGC - 1:
                        attention_group(c)

            # ---------------- Phase 2 tail + o_proj ----------------------
            with tc.tile_pool(name="ph3_wo", bufs=1) as wop, \
                 tc.tile_pool(name="ph3_g", bufs=2) as gp, \
                 tc.tile_pool(name="ph3_o", bufs=3) as oop, \
                 tc.tile_pool(name="ph3_ps", bufs=2, space="PSUM") as p3:
                wo_sb = wop.tile([P, KT, OC], BF16, tag="wo")
                nc.sync.dma_start(out=wo_sb[:], in_=wob[:])
                attention_group(SGC - 1)
                gath_r = gath_d.rearrange("j r h p w -> j p (r h) w")
                for j in range(SGC):
                    g0 = gp.tile([P, 16, CW], BF16, tag="g", name=f"g_{j}_0")
                    nc.sync.dma_start(out=g0[:], in_=gath_r[j, :, 0:16, :])
                    g1 = gp.tile([P, 16, CW], BF16, tag="g", name=f"g_{j}_1")
                    nc.sync.dma_start(out=g1[:], in_=gath_r[j, :, 16:32, :])
                    gh = (g0, g1)
                    for sb in range(4):
                        for oc in range(2):
                            ps = p3.tile([P, CW], F32, tag="o", name=f"o_{j}_{sb}_{oc}")
                            for kt in range(KT):
                                half, i = divmod(kt, 16)
                                nc.tensor.matmul(
                                    ps[:],
                                    gh[half][:, i, sb * P:(sb + 1) * P],
                                    wo_sb[:, kt, oc * CW:(oc + 1) * CW],
                                    start=(kt == 0), stop=(kt == KT - 1),
                                )
                            o_sb = oop.tile([P, CW], F32, tag="osb")
                            nc.vector.tensor_copy(o_sb[:], ps[:])
                            nc.sync.dma_start(
                                out=out[j * CW + sb * P:j * CW + (sb + 1) * P,
                                        oc * CW:(oc + 1) * CW],
                                in_=o_sb[:],
                            )
    nc.compile()
    return nc


def _prep_inputs(hidden_states, w_qkv, w_o, positions):
    """Host-side sharding, bf16 cast and blob tiling. Returns per-core in_maps."""
    hidden_states = np.asarray(hidden_states, dtype=np.float32)
    w_qkv = np.asarray(w_qkv, dtype=np.float32)
    w_o = np.asarray(w_o, dtype=np.float32)
    positions = np.asarray(positions)
    bf16 = ml_dtypes.bfloat16

    # neox permutation of q/k head columns (evens then odds within each head)
    perm = np.concatenate([np.arange(0, D, 2), np.arange(1, D, 2)])
    wq_all = w_qkv[:, :NQ * D].reshape(H, NQ, D)[:, :, perm]      # [H, NQ, D]
    wk_all = w_qkv[:, NQ * D:(NQ + NKV) * D].reshape(H, NKV, D)[:, :, perm]
    wv_all = w_qkv[:, (NQ + NKV) * D:].reshape(H, NKV, D)

    inv_freq = 1.0 / (THETA ** (np.arange(0, D, 2, dtype=np.float32) / D))

    # causal masks for the 4 diagonal 128-row blocks of a 512-wide q group
    kk = np.arange(P)[:, None]
    qq = np.arange(CW)[None, :]
    maskb = np.stack([(kk + r * P <= qq) for r in range(REP)]).astype(bf16)
    maskb = np.ascontiguousarray(maskb.transpose(1, 0, 2))        # [P, REP, CW]

    in_maps = []
    for c in range(NCORES):
        b, t = c // TP, c % TP
        freqs = positions[b].astype(np.float32)[None, :] * inv_freq[:, None]
        cos = np.cos(freqs)                                       # [64, S]
        sin = np.sin(freqs)
        cosf = np.concatenate([cos, cos], axis=0).astype(np.float32)
        sins = np.concatenate([-sin, sin], axis=0).astype(np.float32)

        hidT = hidden_states[b].T                                 # [H, S]
        hidb = np.ascontiguousarray(
            hidT.reshape(KT, P, SGC, CW).transpose(2, 1, 0, 3)
        ).astype(bf16)                                            # [SGC, P, KT, CW]

        wq = wq_all[:, 8 * t:8 * t + 8]                           # [H, 8, 128]
        wk = wk_all[:, 2 * t:2 * t + 2]                           # [H, 2, 128]
        wcols = np.concatenate([wq, wk], axis=1)                  # [H, 10, 128]
        wqkb = np.ascontiguousarray(
            wcols.reshape(KT, P, NCT, P).transpose(2, 1, 0, 3)
        ).astype(bf16)                                            # [NCT, P, KT, P]

        wv = wv_all[:, 2 * t:2 * t + 2].reshape(H, KH * P)
        wvb = np.ascontiguousarray(
            wv.reshape(KT, P, KH * P).transpose(1, 0, 2)
        ).astype(bf16)                                            # [P, KT, 256]

        wo = w_o[:, OC * t:OC * (t + 1)]                          # [H, OC]
        wob = np.ascontiguousarray(
            wo.reshape(KT, P, OC).transpose(1, 0, 2)
        ).astype(bf16)                                            # [P, KT, OC]

        in_maps.append({
            "hidb": hidb, "wqkb": wqkb, "wvb": wvb, "wob": wob,
            "cosf": cosf, "sins": sins, "maskb": maskb,
        })
    return in_maps


_NC_CACHE = {}


def kernel(hidden_states, w_qkv, w_o, positions, _trace=False):
    if "nc" not in _NC_CACHE:
        _NC_CACHE["nc"] = build_program()
    nc = _NC_CACHE["nc"]
    in_maps = _prep_inputs(hidden_states, w_qkv, w_o, positions)
    res = run_bass_kernel_spmd(nc, in_maps, list(range(NCORES)), trace=_trace)
    out_full = np.empty((B, S, H), dtype=np.float32)
    for c in range(NCORES):
        b, t = c // TP, c % TP
        out_full[b, :, OC * t:OC * (t + 1)] = res.results[c]["out"]
    if _trace:
        kernel.last_exec_time_ns = res.exec_time_ns
        kernel.last_profile = res
    return out_full


# revision 9
# speedup vs baseline: 1.3362x; 1.0156x over previous
"""Trainium2 Bass kernel for CohereAttention (B=2, S=2048, H=4096, 32Q/8KV heads, D=128).

Sharding: 8 cores = 2 batch groups x 4 tensor-parallel (head) ranks.
Core c: batch b = c // 4, tp rank t = c % 4; owns q-heads [8t, 8t+8),
kv-heads [2t, 2t+2), w_o column slice [1024t, 1024(t+1)).

v3 notes (on top of the v2 bf16/blob-layout rewrite):
  - all matmul operands bf16 (PSUM f32); host-tiled blob layouts keep DMA
    descriptors 1-64KB.
  - roped qT/kT and v live in SBUF between projection and attention.
  - phase 1 runs in 4 seq chunks of 512; attention group j is emitted
    right after chunk j (group j only needs chunks <= j).
  - softmax denominators for all 8 heads of a group accumulate into rows
    of one [8, 512] PSUM tile; ONE reciprocal per group (the [1,512]
    per-head reciprocal was 3.35us and head-of-line-blocked the Vector
    queue, stalling RoPE evictions and the PE).
  - attention output is evicted unnormalized (frees the PSUM bank), then
    normalized with a select-matrix matmul broadcast of the batched
    reciprocal row; the group finalize (recip/rb/normalize/writes/
    AllGather) is emitted mid-way into the NEXT chunk so its latency
    hides behind projection matmuls.
  - att_d/out writes issue from the Activation engine's HWDGE queue so
    they never head-of-line-block the SP queue carrying the next chunk's
    hid/w loads (this was a 15-18us PE stall per chunk boundary).
  - o_proj pairs the two 512-col output halves per stationary tile into
    one [128, 1024] PSUM tile: half the LDWEIGHTS, one evict + one 4KB-
    line output DMA per seq block.
"""

import numpy as np
import ml_dtypes

import concourse.bass as bass
import concourse.mybir as mybir
from concourse import bacc
import concourse.tile as tile
from concourse.bass_utils import run_bass_kernel_spmd

# Problem constants (fixed by the task).
B, S, H = 2, 2048, 4096
NQ, NKV, D = 32, 8, 128
THETA = 10000.0
NCORES = 8
TP = 4                      # head-parallel group size
QH = NQ // TP               # 8 q heads per core
KH = NKV // TP              # 2 kv heads per core
REP = NQ // NKV             # 4
SCALE = float(D) ** -0.5
P = 128
KT = H // P                 # 32 contraction tiles for projections
OC = H // TP                # 1024 output cols per core
CW = 512                    # seq chunk width
SGC = S // CW               # 4 seq chunks / attention q-groups
NCT = QH + KH               # 10 q/k col-tiles per core
NVB = S // P                # 16 k/v seq blocks
F32 = mybir.dt.float32
BF16 = mybir.dt.bfloat16
RG = [[0, 1, 2, 3], [4, 5, 6, 7]]

Exp = mybir.ActivationFunctionType.Exp


def build_program():
    """Emit the SPMD Bass program (same program for all 8 cores)."""
    nc = bacc.Bacc('TRN2', target_bir_lowering=False, debug=False, num_devices=NCORES)

    # Blob inputs (host pre-tiled; per-partition lines are contiguous in DRAM).
    hidb = nc.dram_tensor("hidb", [SGC, P, KT, CW], BF16, kind="ExternalInput")
    wqkb = nc.dram_tensor("wqkb", [NCT, P, KT, P], BF16, kind="ExternalInput")
    wvb = nc.dram_tensor("wvb", [P, KT, KH * P], BF16, kind="ExternalInput")
    wob = nc.dram_tensor("wob", [P, KT, OC], BF16, kind="ExternalInput")
    cosf = nc.dram_tensor("cosf", [P, S], F32, kind="ExternalInput")
    sins = nc.dram_tensor("sins", [P, S], F32, kind="ExternalInput")
    maskb = nc.dram_tensor("maskb", [P, REP, CW], BF16, kind="ExternalInput")
    selb = nc.dram_tensor("selb", [QH, QH * P], BF16, kind="ExternalInput")
    out = nc.dram_tensor("out", [S, OC], F32, kind="ExternalOutput")

    att_d = nc.dram_tensor("att_d", [SGC, QH, P, CW], BF16)
    gath_d = nc.dram_tensor("gath_d", [SGC, TP, QH, P, CW], BF16)

    with tile.TileContext(nc) as tc:
        with tc.tile_pool(name="persist", bufs=1) as pers, \
             tc.tile_pool(name="att_p", bufs=4) as ppl, \
             tc.tile_pool(name="att_u", bufs=QH + 1) as up, \
             tc.tile_pool(name="att_o", bufs=3) as op, \
             tc.tile_pool(name="ps_s", bufs=2, space="PSUM") as sp, \
             tc.tile_pool(name="ps_att", bufs=2, space="PSUM") as ap_, \
             tc.tile_pool(name="ps_den", bufs=1, space="PSUM") as dp:
            # SBUF-resident across phases.
            qT = pers.tile([P, QH, S], BF16, tag="qT")
            kTt = pers.tile([P, KH, S], BF16, tag="kT")
            vsb = pers.tile([P, NVB, KH * P], BF16, tag="v")
            mask_sb = pers.tile([P, REP, CW], BF16, tag="mask")
            nc.sync.dma_start(out=mask_sb[:], in_=maskb[:])
            sel_sb = pers.tile([QH, QH * P], BF16, tag="sel")
            nc.sync.dma_start(out=sel_sb[:], in_=selb[:])
            ones_c = pers.tile([P, 1], BF16, tag="ones_c")
            nc.vector.memset(ones_c[:], 1.0)

            def attention_compute(j):
                """s/exp/mask/den/att for group j; returns (att_u list, denall)."""
                ncb = 4 * j + 4
                denall = op.tile([QH, CW], F32, tag="denall", name=f"denall_{j}")
                att_us = []
                for h in range(QH):
                    kv = h // REP
                    att_ps = ap_.tile([P, CW], F32, tag="att", name=f"att_{j}_{h}")
                    den_ps = dp.tile([1, CW], F32, tag="den", name=f"den_{j}_{h}")
                    for ci in range(ncb):
                        s_ps = sp.tile([P, CW], F32, tag="s", name=f"s_{j}_{h}_{ci}")
                        nc.tensor.matmul(
                            s_ps[:],
                            kTt[:, kv, ci * P:(ci + 1) * P],
                            qT[:, h, j * CW:(j + 1) * CW],
                            start=True, stop=True,
                        )
                        p_sb = ppl.tile([P, CW], BF16, tag="p", name=f"p_{j}_{h}_{ci}")
                        nc.scalar.activation(p_sb[:], s_ps[:], Exp, scale=SCALE)
                        r = ci - 4 * j
                        if r >= 0:
                            # diagonal chunk: zero where k > q
                            nc.vector.tensor_mul(p_sb[:], p_sb[:], mask_sb[:, r, :])
                        nc.tensor.matmul(
                            den_ps[:], ones_c[:], p_sb[:],
                            start=(ci == 0), stop=(ci == ncb - 1),
                        )
                        nc.tensor.matmul(
                            att_ps[:], vsb[:, ci, kv * P:(kv + 1) * P], p_sb[:],
                            start=(ci == 0), stop=(ci == ncb - 1),
                        )
                    att_u = up.tile([P, CW], BF16, tag="attu", name=f"attu_{j}_{h}")
                    nc.vector.tensor_copy(att_u[:], att_ps[:])
                    # DVE can't write partition base h; bounce via a base-0
                    # tile + SBUF->SBUF DMA (DMA has no partition alignment).
                    den_sb = op.tile([1, CW], F32, tag="den_sb")
                    nc.vector.tensor_copy(den_sb[:], den_ps[:])
                    nc.scalar.dma_start(out=denall[h:h + 1, :], in_=den_sb[:])
                    att_us.append(att_u)
                return att_us, denall

            def attention_finalize(j, att_us, denall):
                """recip/normalize/store/AllGather for group j.

                One batched reciprocal for all 8 heads (the per-head [1,512]
                reciprocal cost the same 3.35us EACH and head-of-line-blocked
                the Vector queue 8x per group).
                """
                rinv8 = op.tile([QH, CW], F32, tag="rinv")
                nc.vector.reciprocal(rinv8[:], denall[:])
                rinv8b = op.tile([QH, CW], BF16, tag="rinvb")
                nc.vector.tensor_copy(rinv8b[:], rinv8[:])
                for h in range(QH):
                    rb_ps = sp.tile([P, CW], F32, tag="s", name=f"rb_{j}_{h}")
                    nc.tensor.matmul(
                        rb_ps[:], sel_sb[:, h * P:(h + 1) * P], rinv8b[:],
                        start=True, stop=True,
                    )
                    rb_sb = op.tile([P, CW], BF16, tag="rb_sb")
                    nc.vector.tensor_copy(rb_sb[:], rb_ps[:])
                    att_sb = op.tile([P, CW], BF16, tag="att_sb")
                    nc.vector.tensor_mul(att_sb[:], att_us[h][:], rb_sb[:])
                    nc.scalar.dma_start(out=att_d[j, h], in_=att_sb[:])
                nc.gpsimd.collective_compute(
                    "AllGather", mybir.AluOpType.bypass,
                    replica_groups=RG,
                    ins=[att_d[j].opt()],
                    outs=[gath_d[j].opt()],
                )

            # ---------------- Phase 1 (+ interleaved attention) ----------
            pending = None
            with tc.tile_pool(name="ph1_hid", bufs=2) as hidp, \
                 tc.tile_pool(name="ph1_w", bufs=2) as wp, \
                 tc.tile_pool(name="ph1_wv", bufs=1) as wvp, \
                 tc.tile_pool(name="ph1_cs", bufs=2) as csp, \
                 tc.tile_pool(name="ph1_rope", bufs=2) as rp, \
                 tc.tile_pool(name="ph1_ps", bufs=2, space="PSUM") as pp, \
                 tc.tile_pool(name="ph1_psv", bufs=1, space="PSUM") as ppv:
                wv_sb = wvp.tile([P, KT, KH * P], BF16, tag="wv")
                nc.sync.dma_start(out=wv_sb[:], in_=wvb[:])
                for c in range(SGC):
                    cs = slice(c * CW, (c + 1) * CW)
                    hid_sb = hidp.tile([P, KT, CW], BF16, tag="hid")
                    nc.sync.dma_start(out=hid_sb[:], in_=hidb[c])
                    cos_sb = csp.tile([P, CW], F32, tag="cos")
                    sin_sb = csp.tile([P, CW], F32, tag="sin")
                    nc.sync.dma_start(out=cos_sb[:], in_=cosf[:, cs])
                    nc.sync.dma_start(out=sin_sb[:], in_=sins[:, cs])
                    for ct in range(NCT):
                        w_sb = wp.tile([P, KT, P], BF16, tag="w")
                        nc.sync.dma_start(out=w_sb[:], in_=wqkb[ct])
                        ps = pp.tile([P, CW], F32, tag="ps", name=f"ps_{c}_{ct}")
                        for kt in range(KT):
                            nc.tensor.matmul(
                                ps[:], w_sb[:, kt, :], hid_sb[:, kt, :],
                                start=(kt == 0), stop=(kt == KT - 1),
                            )
                        # RoPE (neox halves): out = x*cos + rot(x)*sin,
                        # rot(x) = [-x2; x1] with sign folded into sins rows.
                        if ct < QH:
                            dst = qT[:, ct, cs]
                        else:
                            dst = kTt[:, ct - QH, cs]
                        t1 = rp.tile([P, CW], F32, tag="t1")
                        t2 = rp.tile([P, CW], F32, tag="t2")
                        nc.vector.tensor_mul(t1[:], ps[:], cos_sb[:])
                        nc.vector.tensor_mul(t2[0:64, :], ps[64:128, :], sin_sb[0:64, :])
                        nc.vector.tensor_mul(t2[64:128, :], ps[0:64, :], sin_sb[64:128, :])
                        nc.vector.tensor_add(dst, t1[:], t2[:])
                        if ct == 0 and pending is not None:
                            # group (c-1) finalize: its reciprocal/broadcast
                            # latency hides behind this chunk's matmuls.
                            attention_finalize(*pending)
                            pending = None
                    # v projection for this chunk's 4 seq blocks
                    for sbl in range(4):
                        psv = ppv.tile([P, KH * P], F32, tag="psv",
                                       name=f"psv_{c}_{sbl}")
                        for kt in range(KT):
                            nc.tensor.matmul(
                                psv[:],
                                hid_sb[:, kt, sbl * P:(sbl + 1) * P],
                                wv_sb[:, kt, :],
                                start=(kt == 0), stop=(kt == KT - 1),
                            )
                        nc.vector.tensor_copy(vsb[:, c * 4 + sbl, :], psv[:])
                    if c < SGC - 1:
                        pending = (c, *attention_compute(c))

            # ---------------- attention tail -------------------------------
            if pending is not None:
                attention_finalize(*pending)
            attention_finalize(SGC - 1, *attention_compute(SGC - 1))

        # ---------------- o_proj (all prior pools closed) -----------------
        with tc.tile_pool(name="ph3_wo", bufs=1) as wop, \
             tc.tile_pool(name="ph3_g", bufs=3) as gp, \
             tc.tile_pool(name="ph3_o", bufs=2) as oop, \
             tc.tile_pool(name="ph3_ps", bufs=3, space="PSUM") as p3:
            wo_sb = wop.tile([P, KT, OC], BF16, tag="wo")
            nc.sync.dma_start(out=wo_sb[:], in_=wob[:])
            gath_r = gath_d.rearrange("j r h p w -> j p (r h) w")
            for j in range(SGC):
                g0 = gp.tile([P, 16, CW], BF16, tag="g", name=f"g_{j}_0")
                nc.sync.dma_start(out=g0[:], in_=gath_r[j, :, 0:16, :])
                g1 = gp.tile([P, 16, CW], BF16, tag="g", name=f"g_{j}_1")
                nc.sync.dma_start(out=g1[:], in_=gath_r[j, :, 16:32, :])
                gh = (g0, g1)
                for sb in range(4):
                    ps = p3.tile([P, 2 * CW], F32, tag="o", name=f"o_{j}_{sb}")
                    for kt in range(KT):
                        half, i = divmod(kt, 16)
                        g_st = gh[half][:, i, sb * P:(sb + 1) * P]
                        nc.tensor.matmul(
                            ps[:, 0:CW], g_st, wo_sb[:, kt, 0:CW],
                            start=(kt == 0), stop=(kt == KT - 1),
                        )
                        nc.tensor.matmul(
                            ps[:, CW:2 * CW], g_st, wo_sb[:, kt, CW:2 * CW],
                            start=(kt == 0), stop=(kt == KT - 1),
                        )
                    o_sb = oop.tile([P, 2 * CW], F32, tag="osb")
                    nc.vector.tensor_copy(o_sb[:], ps[:])
                    nc.scalar.dma_start(
                        out=out[j * CW + sb * P:j * CW + (sb + 1) * P, :],
                        in_=o_sb[:],
                    )
    nc.compile()
    return nc


def _prep_inputs(hidden_states, w_qkv, w_o, positions):
    """Host-side sharding, bf16 cast and blob tiling. Returns per-core in_maps."""
    hidden_states = np.asarray(hidden_states, dtype=np.float32)
    w_qkv = np.asarray(w_qkv, dtype=np.float32)
    w_o = np.asarray(w_o, dtype=np.float32)
    positions = np.asarray(positions)
    bf16 = ml_dtypes.bfloat16

    # neox permutation of q/k head columns (evens then odds within each head)
    perm = np.concatenate([np.arange(0, D, 2), np.arange(1, D, 2)])
    wq_all = w_qkv[:, :NQ * D].reshape(H, NQ, D)[:, :, perm]      # [H, NQ, D]
    wk_all = w_qkv[:, NQ * D:(NQ + NKV) * D].reshape(H, NKV, D)[:, :, perm]
    wv_all = w_qkv[:, (NQ + NKV) * D:].reshape(H, NKV, D)

    inv_freq = 1.0 / (THETA ** (np.arange(0, D, 2, dtype=np.float32) / D))

    # causal masks for the 4 diagonal 128-row blocks of a 512-wide q group
    kk = np.arange(P)[:, None]
    qq = np.arange(CW)[None, :]
    maskb = np.stack([(kk + r * P <= qq) for r in range(REP)]).astype(bf16)
    maskb = np.ascontiguousarray(maskb.transpose(1, 0, 2))        # [P, REP, CW]

    # head-select matrix for the reciprocal broadcast
    selb = np.zeros((QH, QH * P), dtype=bf16)
    for h in range(QH):
        selb[h, h * P:(h + 1) * P] = 1.0

    in_maps = []
    for c in range(NCORES):
        b, t = c // TP, c % TP
        freqs = positions[b].astype(np.float32)[None, :] * inv_freq[:, None]
        cos = np.cos(freqs)                                       # [64, S]
        sin = np.sin(freqs)
        cosf = np.concatenate([cos, cos], axis=0).astype(np.float32)
        sins = np.concatenate([-sin, sin], axis=0).astype(np.float32)

        hidT = hidden_states[b].T                                 # [H, S]
        hidb = np.ascontiguousarray(
            hidT.reshape(KT, P, SGC, CW).transpose(2, 1, 0, 3)
        ).astype(bf16)                                            # [SGC, P, KT, CW]

        wq = wq_all[:, 8 * t:8 * t + 8]                           # [H, 8, 128]
        wk = wk_all[:, 2 * t:2 * t + 2]                           # [H, 2, 128]
        wcols = np.concatenate([wq, wk], axis=1)                  # [H, 10, 128]
        wqkb = np.ascontiguousarray(
            wcols.reshape(KT, P, NCT, P).transpose(2, 1, 0, 3)
        ).astype(bf16)                                            # [NCT, P, KT, P]

        wv = wv_all[:, 2 * t:2 * t + 2].reshape(H, KH * P)
        wvb = np.ascontiguousarray(
            wv.reshape(KT, P, KH * P).transpose(1, 0, 2)
        ).astype(bf16)                                            # [P, KT, 256]

        wo = w_o[:, OC * t:OC * (t + 1)]                          # [H, OC]
        wob = np.ascontiguousarray(
            wo.reshape(KT, P, OC).transpose(1, 0, 2)
        ).astype(bf16)                                            # [P, KT, OC]

        in_maps.append({
            "hidb": hidb, "wqkb": wqkb, "wvb": wvb, "wob": wob,
            "cosf": cosf, "sins": sins, "maskb": maskb, "selb": selb,
        })
    return in_maps


_NC_CACHE = {}


def kernel(hidden_states, w_qkv, w_o, positions, _trace=False):
    if "nc" not in _NC_CACHE:
        _NC_CACHE["nc"] = build_program()
    nc = _NC_CACHE["nc"]
    in_maps = _prep_inputs(hidden_states, w_qkv, w_o, positions)
    res = run_bass_kernel_spmd(nc, in_maps, list(range(NCORES)), trace=_trace)
    out_full = np.empty((B, S, H), dtype=np.float32)
    for c in range(NCORES):
        b, t = c // TP, c % TP
        out_full[b, :, OC * t:OC * (t + 1)] = res.results[c]["out"]
    if _trace:
        kernel.last_exec_time_ns = res.exec_time_ns
        kernel.last_profile = res
    return out_full
